# revision 9
# baseline (speedup 1.0000x reference)
"""EntityEncoder Trainium2 kernel (8-core SPMD, data-parallel over entities).

Strategy
--------
emb[e] = W_species[sp] + W_ability[ab] + W_item[it]
         + sum_m W_mv[mv_m] + sum_m pp_m * W_pp[mv_m]
         + X_dense[:, e] @ W_small          (level/hp/vol/feats/bool one-hots + bias)
masked (species in {0,1}) rows -> 0.

Device work per core (8192 entities), row-major layout (entity -> partition):
 - 11 dma_gather units (fp16 tables in HBM): species, ability, item,
   4 move-halves, 4 pp-halves.
 - PE: dense block as 2 K-tile matmul into PSUM + identity-matmul
   accumulation of move-halves and the DVE accumulator into PSUM.
 - DVE: species+ability+item adds, pp scaled-adds (scalar_tensor_tensor).
 - ACT: PSUM drain + cast to fp16.
 - SWDGE casting DMA writes the fp32 output.

Host prep only reshapes/encodes ints (index packing, one-hot layout) and
type-converts weights; all FLOPs involving weights x activations run on
device. Masking is folded into the indices (zero-row redirect).
"""

import os
import numpy as np

os.environ.setdefault("MYCRO_LOCAL_CACHE", "1")

import concourse.bass as bass
import concourse.mybir as mybir
import concourse.tile as tile
from concourse import bacc
from concourse.bass_utils import run_bass_kernel_spmd

# ---------------- problem constants (hardcoded per spec) ----------------
NUM_SPECIES, NUM_ABILITIES, NUM_ITEMS, NUM_MOVES = 1280, 320, 1024, 896
NUM_GENDERS, NUM_STATUS = 4, 8
D = 512
N = 65536
N_CORES = 8
E = N // N_CORES  # 8192 entities per core

SPECIES, ABILITY, ITEM, LEVEL, HP = 0, 1, 2, 3, 4
GENDER, STATUS, CALLED_BACK, TRAPPED, NEWLY_SW = 5, 6, 7, 8, 9
TOXIC, SLEEP, FAINTED, ACTIVE = 10, 11, 12, 13
BOOST0 = 14
MOVEID0, MOVEPP0, HAS_STATUS = 21, 25, 29
VOL0, VOL_END = 30, 39

K_DENSE = 188  # 7 level bits + 10 hp bits + 36 vol bits + 9 feats + 125 bool + 1 bias
K_HI = 128
K_LO = K_DENSE - K_HI  # 60

# ---------------- tunables ----------------
CHUNK = 1024              # entities per chunk (8 groups of 128)
GROUPS = CHUNK // 128     # 8
N_CHUNKS = E // CHUNK     # 8
F16 = mybir.dt.float16
F32 = mybir.dt.float32
I16 = mybir.dt.int16

_NC_CACHE = {}
LAST_RESULTS = None  # BassKernelResults of the most recent kernel() call


# ============================ host-side prep ============================

def _bits_np(x, nbits):
    return ((x[..., None] >> np.arange(nbits)) & 1).astype(np.float32)


def _build_tables(inp):
    """fp16 gather tables with a trailing zero row, and dense-block weights."""
    z = np.zeros((1, D), np.float32)
    t_sp = np.concatenate([inp["W_species"], z], 0).astype(np.float16)
    t_ab = np.concatenate([inp["W_ability"], z], 0).astype(np.float16)
    t_it = np.concatenate([inp["W_item"], z], 0).astype(np.float16)
    W_mv = inp["W_moveset"][:NUM_MOVES]
    W_pp = inp["W_moveset"][NUM_MOVES:]
    t_mv = np.concatenate(
        [np.concatenate([W_mv, W_pp], 1),
         np.zeros((1, 2 * D), np.float32)], 0).astype(np.float16)

    bias = (inp["b_species"] + inp["b_ability"] + inp["b_item"] + inp["b_moveset"]
            + inp["b_level"] + inp["b_hp"] + inp["b_volatiles"] + inp["b_feats"]
            + inp["b_bool"]).astype(np.float32)
    w_small = np.concatenate(
        [inp["W_level"], inp["W_hp"], inp["W_volatiles"], inp["W_feats"],
         inp["W_bool"], bias[None]], 0).astype(np.float16)
    assert w_small.shape == (K_DENSE, D)
    return t_sp, t_ab, t_it, t_mv, w_small


def _build_x_dense(e, mask):
    """[K_DENSE, n] fp16 dense-block encodings; masked columns zeroed."""
    n = e.shape[0]
    rows = [
        _bits_np(e[:, LEVEL], 7).T,                      # 7
        _bits_np(e[:, HP], 10).T,                        # 10
        _bits_np(e[:, VOL0:VOL_END], 4).reshape(n, 36).T,  # 36
        np.stack([e[:, LEVEL] / 100.0, e[:, HP] / 1023.0], 0).astype(np.float32),  # 2
        (e[:, BOOST0:BOOST0 + 7].T * 0.5).astype(np.float32),  # 7
    ]
    oh = np.zeros((125, n), np.float32)
    off = 0
    for col, k in [(GENDER, 4), (STATUS, 8), (CALLED_BACK, 2), (TRAPPED, 2),
                   (NEWLY_SW, 2), (TOXIC, 8), (SLEEP, 4), (FAINTED, 2), (ACTIVE, 2)]:
        oh[off + e[:, col], np.arange(n)] = 1.0
        off += k
    for i in range(7):
        oh[off + e[:, BOOST0 + i] + 6, np.arange(n)] = 1.0
        off += 13
    assert off == 125
    rows.append(oh)
    rows.append(np.ones((1, n), np.float32))
    x = np.concatenate(rows, 0)
    x[:, ~mask] = 0.0
    return x.astype(np.float16)


def _wrap_idx(idx):
    """[n] -> [128, n/16] wrapped layout: entry [p%16, s] = idx[s*16+p%16],
    replicated across the 8 16-partition stripes (one per Q7 core)."""
    w = np.ascontiguousarray(idx.astype(np.int16).reshape(-1, 16).T)
    return np.tile(w, (8, 1))


def _prep_core(e, mask):
    """Per-core derived arrays for one shard (e: [n, 39] int32)."""
    n = e.shape[0]
    zr_sp, zr_ab, zr_it, zr_mv = NUM_SPECIES, NUM_ABILITIES, NUM_ITEMS, NUM_MOVES
    sp = np.where(mask, e[:, SPECIES], zr_sp)
    ab = np.where(mask, e[:, ABILITY], zr_ab)
    it = np.where(mask, e[:, ITEM], zr_it)
    mv = np.where(mask[:, None], e[:, MOVEID0:MOVEPP0], zr_mv)  # [n,4]
    pp = np.where(mask[:, None], e[:, MOVEPP0:HAS_STATUS] / 1023.0, 0.0)  # [n,4]

    idx_sp = _wrap_idx(sp)
    idx_ab = _wrap_idx(ab)
    idx_it = _wrap_idx(it)
    # one tensor [16, 4*n/16]; move m occupies columns [m*n/16, (m+1)*n/16)
    idx_mv = np.concatenate([_wrap_idx(mv[:, m]) for m in range(4)], 1)
    # pp scalars: [128, 4, n/128], entry [p, m, G] = pp[G*128+p, m]
    pp_sc = np.ascontiguousarray(
        pp.reshape(n // 128, 128, 4).transpose(1, 2, 0)).astype(np.float32)
    x = _build_x_dense(e, mask)  # [188, E]
    return dict(idx_sp=idx_sp, idx_ab=idx_ab, idx_it=idx_it, idx_mv=idx_mv,
                pp_sc=pp_sc, x_hi=np.ascontiguousarray(x[:K_HI]),
                x_lo=np.ascontiguousarray(x[K_HI:]))


# ============================ bass program ============================

def _build_nc(n_ent):
    """SPMD program for one core processing n_ent entities."""
    n_chunks = n_ent // CHUNK
    nc = bacc.Bacc("TRN2", target_bir_lowering=False, debug=False)

    t_sp = nc.dram_tensor("t_sp", [NUM_SPECIES + 1, D], F16, kind="ExternalInput")
    t_ab = nc.dram_tensor("t_ab", [NUM_ABILITIES + 1, D], F16, kind="ExternalInput")
    t_it = nc.dram_tensor("t_it", [NUM_ITEMS + 1, D], F16, kind="ExternalInput")
    t_mv = nc.dram_tensor("t_mv", [NUM_MOVES + 1, 2 * D], F16, kind="ExternalInput")
    w_hi_d = nc.dram_tensor("w_hi", [K_HI, D], F16, kind="ExternalInput")
    w_lo_d = nc.dram_tensor("w_lo", [K_LO, D], F16, kind="ExternalInput")
    x_hi_d = nc.dram_tensor("x_hi", [K_HI, n_ent], F16, kind="ExternalInput")
    x_lo_d = nc.dram_tensor("x_lo", [K_LO, n_ent], F16, kind="ExternalInput")
    idx_sp_d = nc.dram_tensor("idx_sp", [128, n_ent // 16], I16, kind="ExternalInput")
    idx_ab_d = nc.dram_tensor("idx_ab", [128, n_ent // 16], I16, kind="ExternalInput")
    idx_it_d = nc.dram_tensor("idx_it", [128, n_ent // 16], I16, kind="ExternalInput")
    idx_mv_d = nc.dram_tensor("idx_mv", [128, 4 * n_ent // 16], I16, kind="ExternalInput")
    pp_sc_d = nc.dram_tensor("pp_sc", [128, 4, n_ent // 128], F32, kind="ExternalInput")
    out_d = nc.dram_tensor("out", [n_ent, D], F32, kind="ExternalOutput")

    # dram view [chunk, p, g, d] matching gather partition layout
    out_r = out_d.ap().rearrange("(c g p) d -> c p g d", p=128, g=GROUPS)

    scol = n_ent // 16  # wrapped-idx columns per unit

    with tile.TileContext(nc) as tc:
        with (
            tc.tile_pool(name="consts", bufs=1) as pc,
            tc.tile_pool(name="gather", bufs=1) as pg,
            tc.tile_pool(name="accp", bufs=2) as pa,
            tc.tile_pool(name="outp", bufs=2) as po,
            tc.tile_pool(name="psum", bufs=8, space="PSUM") as pp_pool,
        ):
            # ---- constants into SBUF ----
            w_hi = pc.tile([K_HI, D], F16, tag="w_hi")
            w_lo = pc.tile([K_LO, D], F16, tag="w_lo")
            x_hi = pc.tile([K_HI, n_ent], F16, tag="x_hi")
            x_lo = pc.tile([K_LO, n_ent], F16, tag="x_lo")
            idx_sp = pc.tile([128, scol], I16, tag="idx_sp")
            idx_ab = pc.tile([128, scol], I16, tag="idx_ab")
            idx_it = pc.tile([128, scol], I16, tag="idx_it")
            idx_mv = pc.tile([128, 4 * scol], I16, tag="idx_mv")
            pp_sc = pc.tile([128, 4, n_ent // 128], F32, tag="pp_sc")
            ident = pc.tile([128, 128], F16, tag="ident")

            nc.sync.dma_start(w_hi[:], w_hi_d.ap())
            nc.sync.dma_start(w_lo[:], w_lo_d.ap())
            nc.sync.dma_start(x_hi[:], x_hi_d.ap())
            nc.sync.dma_start(x_lo[:], x_lo_d.ap())
            nc.sync.dma_start(idx_sp[:], idx_sp_d.ap())
            nc.sync.dma_start(idx_ab[:], idx_ab_d.ap())
            nc.sync.dma_start(idx_it[:], idx_it_d.ap())
            nc.sync.dma_start(idx_mv[:], idx_mv_d.ap())
            nc.sync.dma_start(pp_sc[:], pp_sc_d.ap())

            nc.gpsimd.memset(ident[:], 0.0)
            nc.gpsimd.affine_select(
                out=ident[:], in_=ident[:],
                compare_op=mybir.AluOpType.not_equal,
                fill=1.0, base=0, pattern=[[-1, 128]], channel_multiplier=1,
            )

            ccol = CHUNK // 16  # wrapped-idx columns per chunk

            for c in range(n_chunks):
                isl = slice(c * ccol, (c + 1) * ccol)

                def gath(dst, table_ap, idx_ap, estep):
                    nc.gpsimd.dma_gather(
                        dst[:], table_ap, idx_ap, CHUNK, CHUNK, D,
                        elem_step=estep)

                g_sp = pg.tile([128, GROUPS, D], F16, tag="g_sp")
                gath(g_sp, t_sp.ap(), idx_sp[:, isl], D)
                g_ab = pg.tile([128, GROUPS, D], F16, tag="g_ab")
                gath(g_ab, t_ab.ap(), idx_ab[:, isl], D)
                g_it = pg.tile([128, GROUPS, D], F16, tag="g_it")
                gath(g_it, t_it.ap(), idx_it[:, isl], D)
                g_mv = []
                g_pp = []
                for m in range(4):
                    misl = slice(m * scol + c * ccol, m * scol + (c + 1) * ccol)
                    gm = pg.tile([128, GROUPS, D], F16, tag=f"g_mv{m}")
                    gath(gm, t_mv.ap()[:, 0:D], idx_mv[:, misl], 2 * D)
                    g_mv.append(gm)
                    gp = pg.tile([128, GROUPS, D], F16, tag=f"g_pp{m}")
                    gath(gp, t_mv.ap()[:, D:2 * D], idx_mv[:, misl], 2 * D)
                    g_pp.append(gp)

                # ---- DVE accumulator: species + ability + item + pp terms ----
                acc = pa.tile([128, GROUPS, D], F16, tag="acc")
                nc.vector.tensor_tensor(acc[:], g_sp[:], g_ab[:], mybir.AluOpType.add)
                nc.vector.tensor_tensor(acc[:], acc[:], g_it[:], mybir.AluOpType.add)
                for m in range(4):
                    for g in range(GROUPS):
                        G = c * GROUPS + g
                        nc.vector.scalar_tensor_tensor(
                            acc[:, g, :], g_pp[m][:, g, :],
                            pp_sc[:, m, G:G + 1], acc[:, g, :],
                            mybir.AluOpType.mult, mybir.AluOpType.add)

                # ---- PE: dense block + move-halves + acc into PSUM ----
                out_sb = po.tile([128, GROUPS, D], F16, tag="out_sb")
                for g in range(GROUPS):
                    ecol = c * CHUNK + g * 128
                    ps = pp_pool.tile([128, D], F32, tag="ps")
                    nc.tensor.matmul(ps[:], x_hi[:, ecol:ecol + 128], w_hi[:],
                                     start=True, stop=False)
                    nc.tensor.matmul(ps[:], x_lo[:, ecol:ecol + 128], w_lo[:],
                                     start=False, stop=False)
                    for m in range(4):
                        nc.tensor.matmul(ps[:], ident[:], g_mv[m][:, g, :],
                                         start=False, stop=False)
                    nc.tensor.matmul(ps[:], ident[:], acc[:, g, :],
                                     start=False, stop=True)
                    nc.scalar.activation(out_sb[:, g, :], ps[:],
                                         mybir.ActivationFunctionType.Copy)

                nc.gpsimd.dma_start(out_r[c], out_sb[:])

    nc.compile()
    return nc


# ============================ entry point ============================

def _get_nc(n_ent):
    if n_ent not in _NC_CACHE:
        _NC_CACHE[n_ent] = _build_nc(n_ent)
    return _NC_CACHE[n_ent]


def kernel(**inputs):
    entity = np.asarray(inputs["entity"])
    mask_full = ~((entity[:, SPECIES] == 0) | (entity[:, SPECIES] == 1))

    t_sp, t_ab, t_it, t_mv, w_small = _build_tables(
        {k: np.asarray(v) for k, v in inputs.items()})
    shared = dict(t_sp=t_sp, t_ab=t_ab, t_it=t_it, t_mv=t_mv,
                  w_hi=np.ascontiguousarray(w_small[:K_HI]),
                  w_lo=np.ascontiguousarray(w_small[K_HI:]))

    in_maps = []
    for c in range(N_CORES):
        sl = slice(c * E, (c + 1) * E)
        per = _prep_core(entity[sl], mask_full[sl])
        in_maps.append({**shared, **per})

    nc = _get_nc(E)
    res = run_bass_kernel_spmd(nc, in_maps, list(range(N_CORES)))
    global LAST_RESULTS
    LAST_RESULTS = res
    emb = np.concatenate([r["out"] for r in res.results], 0)
    return emb.astype(np.float32), mask_full


# revision 18
# speedup vs baseline: 1.4814x; 1.4814x over previous
"""EntityEncoder Trainium2 kernel (8-core SPMD, data-parallel over entities).

Strategy
--------
emb[e] = W_species[sp] + W_ability[ab] + W_item[it]
         + sum_m W_mv[mv_m] + sum_m pp_m * W_pp[mv_m]
         + X_dense[:, e] @ W_small          (level/hp/vol/feats/bool one-hots + bias)
masked (species in {0,1}) rows -> 0.

Device work per core (8192 entities), row-major layout (entity -> partition):
 - 11 dma_gather units (fp16 tables in HBM): species, ability, item,
   4 move-halves, 4 pp-halves.
 - PE: dense block as 2 K-tile matmul into PSUM + identity-matmul
   accumulation of move-halves and the DVE accumulator into PSUM.
 - DVE: species+ability+item adds, pp scaled-adds (scalar_tensor_tensor).
 - ACT: PSUM drain + cast to fp16.
 - SWDGE casting DMA writes the fp32 output.

Host prep only reshapes/encodes ints (index packing, one-hot layout) and
type-converts weights; all FLOPs involving weights x activations run on
device. Masking is folded into the indices (zero-row redirect).
"""

import os
import numpy as np

os.environ.setdefault("MYCRO_LOCAL_CACHE", "1")

import concourse.bass as bass
import concourse.mybir as mybir
import concourse.tile as tile
from concourse import bacc
from concourse.bass_utils import run_bass_kernel_spmd

# ---------------- problem constants (hardcoded per spec) ----------------
NUM_SPECIES, NUM_ABILITIES, NUM_ITEMS, NUM_MOVES = 1280, 320, 1024, 896
NUM_GENDERS, NUM_STATUS = 4, 8
D = 512
N = 65536
N_CORES = 8
E = N // N_CORES  # 8192 entities per core

SPECIES, ABILITY, ITEM, LEVEL, HP = 0, 1, 2, 3, 4
GENDER, STATUS, CALLED_BACK, TRAPPED, NEWLY_SW = 5, 6, 7, 8, 9
TOXIC, SLEEP, FAINTED, ACTIVE = 10, 11, 12, 13
BOOST0 = 14
MOVEID0, MOVEPP0, HAS_STATUS = 21, 25, 29
VOL0, VOL_END = 30, 39

K_DENSE = 188  # 7 level bits + 10 hp bits + 36 vol bits + 9 feats + 125 bool + 1 bias
K_HI = 128
K_LO = K_DENSE - K_HI  # 60

# ---------------- tunables ----------------
CHUNK = 1024              # entities per chunk (8 groups of 128)
GROUPS = CHUNK // 128     # 8
N_CHUNKS = E // CHUNK     # 8
F16 = mybir.dt.float16
F32 = mybir.dt.float32
I16 = mybir.dt.int16

_NC_CACHE = {}
LAST_RESULTS = None  # BassKernelResults of the most recent kernel() call


# ============================ host-side prep ============================

def _bits_np(x, nbits):
    return ((x[..., None] >> np.arange(nbits)) & 1).astype(np.float32)


def _build_tables(inp):
    """fp16 gather tables with a trailing zero row, and dense-block weights."""
    z = np.zeros((1, D), np.float32)
    t_sp = np.concatenate([inp["W_species"], z], 0).astype(np.float16)
    t_ab = np.concatenate([inp["W_ability"], z], 0).astype(np.float16)
    t_it = np.concatenate([inp["W_item"], z], 0).astype(np.float16)
    W_mv = inp["W_moveset"][:NUM_MOVES]
    W_pp = inp["W_moveset"][NUM_MOVES:]
    t_mv = np.concatenate(
        [np.concatenate([W_mv, W_pp], 1),
         np.zeros((1, 2 * D), np.float32)], 0).astype(np.float16)

    bias = (inp["b_species"] + inp["b_ability"] + inp["b_item"] + inp["b_moveset"]
            + inp["b_level"] + inp["b_hp"] + inp["b_volatiles"] + inp["b_feats"]
            + inp["b_bool"]).astype(np.float32)
    w_small = np.concatenate(
        [inp["W_level"], inp["W_hp"], inp["W_volatiles"], inp["W_feats"],
         inp["W_bool"], bias[None]], 0).astype(np.float16)
    assert w_small.shape == (K_DENSE, D)
    return t_sp, t_ab, t_it, t_mv, w_small


def _build_x_dense(e, mask):
    """[K_DENSE, n] fp16 dense-block encodings; masked columns zeroed."""
    n = e.shape[0]
    rows = [
        _bits_np(e[:, LEVEL], 7).T,                      # 7
        _bits_np(e[:, HP], 10).T,                        # 10
        _bits_np(e[:, VOL0:VOL_END], 4).reshape(n, 36).T,  # 36
        np.stack([e[:, LEVEL] / 100.0, e[:, HP] / 1023.0], 0).astype(np.float32),  # 2
        (e[:, BOOST0:BOOST0 + 7].T * 0.5).astype(np.float32),  # 7
    ]
    oh = np.zeros((125, n), np.float32)
    off = 0
    for col, k in [(GENDER, 4), (STATUS, 8), (CALLED_BACK, 2), (TRAPPED, 2),
                   (NEWLY_SW, 2), (TOXIC, 8), (SLEEP, 4), (FAINTED, 2), (ACTIVE, 2)]:
        oh[off + e[:, col], np.arange(n)] = 1.0
        off += k
    for i in range(7):
        oh[off + e[:, BOOST0 + i] + 6, np.arange(n)] = 1.0
        off += 13
    assert off == 125
    rows.append(oh)
    rows.append(np.ones((1, n), np.float32))
    x = np.concatenate(rows, 0)
    x[:, ~mask] = 0.0
    return x.astype(np.float16)


def _wrap_idx(idx):
    """[n] -> [128, n/16] wrapped layout: entry [p%16, s] = idx[s*16+p%16],
    replicated across the 8 16-partition stripes (one per Q7 core)."""
    w = np.ascontiguousarray(idx.astype(np.int16).reshape(-1, 16).T)
    return np.tile(w, (8, 1))


def _prep_core(e, mask):
    """Per-core derived arrays for one shard (e: [n, 39] int32)."""
    n = e.shape[0]
    zr_sp, zr_ab, zr_it, zr_mv = NUM_SPECIES, NUM_ABILITIES, NUM_ITEMS, NUM_MOVES
    sp = np.where(mask, e[:, SPECIES], zr_sp)
    ab = np.where(mask, e[:, ABILITY], zr_ab)
    it = np.where(mask, e[:, ITEM], zr_it)
    mv = np.where(mask[:, None], e[:, MOVEID0:MOVEPP0], zr_mv)  # [n,4]
    pp = np.where(mask[:, None], e[:, MOVEPP0:HAS_STATUS] / 1023.0, 0.0)  # [n,4]

    idx_sp = _wrap_idx(sp)
    idx_ab = _wrap_idx(ab)
    idx_it = _wrap_idx(it)
    # one tensor [16, 4*n/16]; move m occupies columns [m*n/16, (m+1)*n/16)
    idx_mv = np.concatenate([_wrap_idx(mv[:, m]) for m in range(4)], 1)
    # pp scalars: [128, 4, n/128], entry [p, m, G] = pp[G*128+p, m]
    pp_sc = np.ascontiguousarray(
        pp.reshape(n // 128, 128, 4).transpose(1, 2, 0)).astype(np.float32)
    x = _build_x_dense(e, mask)  # [188, E]
    return dict(idx_sp=idx_sp, idx_ab=idx_ab, idx_it=idx_it, idx_mv=idx_mv,
                pp_sc=pp_sc, x_hi=np.ascontiguousarray(x[:K_HI]),
                x_lo=np.ascontiguousarray(x[K_HI:]))


# ============================ bass program ============================

def _build_nc(n_ent):
    """SPMD program for one core processing n_ent entities."""
    n_chunks = n_ent // CHUNK
    nc = bacc.Bacc("TRN2", target_bir_lowering=False, debug=False)

    t_sp = nc.dram_tensor("t_sp", [NUM_SPECIES + 1, D], F16, kind="ExternalInput")
    t_ab = nc.dram_tensor("t_ab", [NUM_ABILITIES + 1, D], F16, kind="ExternalInput")
    t_it = nc.dram_tensor("t_it", [NUM_ITEMS + 1, D], F16, kind="ExternalInput")
    t_mv = nc.dram_tensor("t_mv", [NUM_MOVES + 1, 2 * D], F16, kind="ExternalInput")
    w_hi_d = nc.dram_tensor("w_hi", [K_HI, D], F16, kind="ExternalInput")
    w_lo_d = nc.dram_tensor("w_lo", [K_LO, D], F16, kind="ExternalInput")
    x_hi_d = nc.dram_tensor("x_hi", [K_HI, n_ent], F16, kind="ExternalInput")
    x_lo_d = nc.dram_tensor("x_lo", [K_LO, n_ent], F16, kind="ExternalInput")
    idx_sp_d = nc.dram_tensor("idx_sp", [128, n_ent // 16], I16, kind="ExternalInput")
    idx_ab_d = nc.dram_tensor("idx_ab", [128, n_ent // 16], I16, kind="ExternalInput")
    idx_it_d = nc.dram_tensor("idx_it", [128, n_ent // 16], I16, kind="ExternalInput")
    idx_mv_d = nc.dram_tensor("idx_mv", [128, 4 * n_ent // 16], I16, kind="ExternalInput")
    pp_sc_d = nc.dram_tensor("pp_sc", [128, 4, n_ent // 128], F32, kind="ExternalInput")
    out_d = nc.dram_tensor("out", [n_ent, D], F32, kind="ExternalOutput")

    # dram view [chunk, p, g, d] matching gather partition layout
    out_r = out_d.ap().rearrange("(c g p) d -> c p g d", p=128, g=GROUPS)

    scol = n_ent // 16  # wrapped-idx columns per unit

    with tile.TileContext(nc) as tc:
        with (
            tc.tile_pool(name="consts", bufs=1) as pc,
            tc.tile_pool(name="gather", bufs=1) as pg,
            tc.tile_pool(name="accp", bufs=2) as pa,
            tc.tile_pool(name="outp", bufs=2) as po,
            tc.tile_pool(name="psum", bufs=8, space="PSUM") as pp_pool,
        ):
            # ---- constants into SBUF ----
            w_hi = pc.tile([K_HI, D], F16, tag="w_hi")
            w_lo = pc.tile([K_LO, D], F16, tag="w_lo")
            x_hi = pc.tile([K_HI, n_ent], F16, tag="x_hi")
            x_lo = pc.tile([K_LO, n_ent], F16, tag="x_lo")
            idx_sp = pc.tile([128, scol], I16, tag="idx_sp")
            idx_ab = pc.tile([128, scol], I16, tag="idx_ab")
            idx_it = pc.tile([128, scol], I16, tag="idx_it")
            idx_mv = pc.tile([128, 4 * scol], I16, tag="idx_mv")
            pp_sc = pc.tile([128, 4, n_ent // 128], F32, tag="pp_sc")
            ident = pc.tile([128, 128], F16, tag="ident")

            nc.sync.dma_start(w_hi[:], w_hi_d.ap())
            nc.sync.dma_start(w_lo[:], w_lo_d.ap())
            nc.sync.dma_start(x_hi[:], x_hi_d.ap())
            nc.sync.dma_start(x_lo[:], x_lo_d.ap())
            nc.sync.dma_start(idx_sp[:], idx_sp_d.ap())
            nc.sync.dma_start(idx_ab[:], idx_ab_d.ap())
            nc.sync.dma_start(idx_it[:], idx_it_d.ap())
            nc.sync.dma_start(idx_mv[:], idx_mv_d.ap())
            nc.sync.dma_start(pp_sc[:], pp_sc_d.ap())

            nc.gpsimd.memset(ident[:], 0.0)
            nc.gpsimd.affine_select(
                out=ident[:], in_=ident[:],
                compare_op=mybir.AluOpType.not_equal,
                fill=1.0, base=0, pattern=[[-1, 128]], channel_multiplier=1,
            )

            ccol = CHUNK // 16  # wrapped-idx columns per chunk

            for c in range(n_chunks):
                isl = slice(c * ccol, (c + 1) * ccol)

                def gath(dst, table_ap, idx_ap, estep):
                    nc.gpsimd.dma_gather(
                        dst[:], table_ap, idx_ap, CHUNK, CHUNK, D,
                        elem_step=estep)

                g_sp = pg.tile([128, GROUPS, D], F16, tag="g_sp")
                gath(g_sp, t_sp.ap(), idx_sp[:, isl], D)
                g_ab = pg.tile([128, GROUPS, D], F16, tag="g_ab")
                gath(g_ab, t_ab.ap(), idx_ab[:, isl], D)
                g_it = pg.tile([128, GROUPS, D], F16, tag="g_it")
                gath(g_it, t_it.ap(), idx_it[:, isl], D)
                g_mv = []
                g_pp = []
                for m in range(4):
                    misl = slice(m * scol + c * ccol, m * scol + (c + 1) * ccol)
                    gq = pg.tile([128, GROUPS, 2 * D], F16, tag=f"g_q{m}")
                    nc.gpsimd.dma_gather(
                        gq[:], t_mv.ap(), idx_mv[:, misl], CHUNK, CHUNK,
                        2 * D, elem_step=2 * D)
                    g_mv.append(gq[:, :, 0:D])
                    g_pp.append(gq[:, :, D:2 * D])

                # ---- DVE accumulator: species + ability + item + pp terms ----
                acc = pa.tile([128, GROUPS, D], F16, tag="acc")
                nc.vector.tensor_tensor(acc[:], g_sp[:], g_ab[:], mybir.AluOpType.add)
                nc.vector.tensor_tensor(acc[:], acc[:], g_it[:], mybir.AluOpType.add)
                for m in range(4):
                    for g in range(GROUPS):
                        G = c * GROUPS + g
                        nc.vector.scalar_tensor_tensor(
                            acc[:, g, :], g_pp[m][:, g, :],
                            pp_sc[:, m, G:G + 1], acc[:, g, :],
                            mybir.AluOpType.mult, mybir.AluOpType.add)

                # ---- PE: dense block + move-halves + acc into PSUM ----
                out_sb = po.tile([128, GROUPS, D], F16, tag="out_sb")
                for g in range(GROUPS):
                    ecol = c * CHUNK + g * 128
                    ps = pp_pool.tile([128, D], F32, tag="ps")
                    nc.tensor.matmul(ps[:], x_hi[:, ecol:ecol + 128], w_hi[:],
                                     start=True, stop=False)
                    nc.tensor.matmul(ps[:], x_lo[:, ecol:ecol + 128], w_lo[:],
                                     start=False, stop=False)
                    for m in range(4):
                        nc.tensor.matmul(ps[:], ident[:], g_mv[m][:, g, :],
                                         start=False, stop=False)
                    nc.tensor.matmul(ps[:], ident[:], acc[:, g, :],
                                     start=False, stop=True)
                    nc.scalar.activation(out_sb[:, g, :], ps[:],
                                         mybir.ActivationFunctionType.Copy)

                nc.gpsimd.dma_start(out_r[c], out_sb[:])

    nc.compile()
    return nc


# ============================ entry point ============================

def _get_nc(n_ent):
    if n_ent not in _NC_CACHE:
        _NC_CACHE[n_ent] = _build_nc(n_ent)
    return _NC_CACHE[n_ent]


def kernel(**inputs):
    entity = np.asarray(inputs["entity"])
    mask_full = ~((entity[:, SPECIES] == 0) | (entity[:, SPECIES] == 1))

    t_sp, t_ab, t_it, t_mv, w_small = _build_tables(
        {k: np.asarray(v) for k, v in inputs.items()})
    shared = dict(t_sp=t_sp, t_ab=t_ab, t_it=t_it, t_mv=t_mv,
                  w_hi=np.ascontiguousarray(w_small[:K_HI]),
                  w_lo=np.ascontiguousarray(w_small[K_HI:]))

    in_maps = []
    for c in range(N_CORES):
        sl = slice(c * E, (c + 1) * E)
        per = _prep_core(entity[sl], mask_full[sl])
        in_maps.append({**shared, **per})

    nc = _get_nc(E)
    res = run_bass_kernel_spmd(nc, in_maps, list(range(N_CORES)))
    global LAST_RESULTS
    LAST_RESULTS = res
    emb = np.concatenate([r["out"] for r in res.results], 0)
    return emb.astype(np.float32), mask_full


# revision 39
# speedup vs baseline: 1.4867x; 1.0035x over previous
"""EntityEncoder Trainium2 kernel (8-core SPMD, data-parallel over entities).

Strategy
--------
emb[e] = W_species[sp] + W_ability[ab] + W_item[it]
         + sum_m W_mv[mv_m] + sum_m pp_m * W_pp[mv_m]
         + X_dense[:, e] @ W_small          (level/hp/vol/feats/bool one-hots + bias)
masked (species in {0,1}) rows -> 0.

Device work per core (8192 entities), row-major layout (entity -> partition):
 - 11 dma_gather units (fp16 tables in HBM): species, ability, item,
   4 move-halves, 4 pp-halves.
 - PE: dense block as 2 K-tile matmul into PSUM + identity-matmul
   accumulation of move-halves and the DVE accumulator into PSUM.
 - DVE: species+ability+item adds, pp scaled-adds (scalar_tensor_tensor).
 - ACT: PSUM drain + cast to fp16.
 - SWDGE casting DMA writes the fp32 output.

Host prep only reshapes/encodes ints (index packing, one-hot layout) and
type-converts weights; all FLOPs involving weights x activations run on
device. Masking is folded into the indices (zero-row redirect).
"""

import os
import numpy as np

os.environ.setdefault("MYCRO_LOCAL_CACHE", "1")

import concourse.bass as bass
import concourse.mybir as mybir
import concourse.tile as tile
from concourse import bacc
from concourse.bass_utils import run_bass_kernel_spmd

# ---------------- problem constants (hardcoded per spec) ----------------
NUM_SPECIES, NUM_ABILITIES, NUM_ITEMS, NUM_MOVES = 1280, 320, 1024, 896
NUM_GENDERS, NUM_STATUS = 4, 8
D = 512
N = 65536
N_CORES = 8
E = N // N_CORES  # 8192 entities per core

SPECIES, ABILITY, ITEM, LEVEL, HP = 0, 1, 2, 3, 4
GENDER, STATUS, CALLED_BACK, TRAPPED, NEWLY_SW = 5, 6, 7, 8, 9
TOXIC, SLEEP, FAINTED, ACTIVE = 10, 11, 12, 13
BOOST0 = 14
MOVEID0, MOVEPP0, HAS_STATUS = 21, 25, 29
VOL0, VOL_END = 30, 39

K_DENSE = 188  # 7 level bits + 10 hp bits + 36 vol bits + 9 feats + 125 bool + 1 bias
K_HI = 128
K_LO = K_DENSE - K_HI  # 60

# ---------------- tunables ----------------
CHUNK = 1024              # entities per chunk (8 groups of 128)
GROUPS = CHUNK // 128     # 8
N_CHUNKS = E // CHUNK     # 8
F16 = mybir.dt.float16
F32 = mybir.dt.float32
I16 = mybir.dt.int16

_NC_CACHE = {}
LAST_RESULTS = None  # BassKernelResults of the most recent kernel() call


# ============================ host-side prep ============================

def _bits_np(x, nbits):
    return ((x[..., None] >> np.arange(nbits)) & 1).astype(np.float32)


def _build_tables(inp):
    """fp16 gather tables with a trailing zero row, and dense-block weights."""
    z = np.zeros((1, D), np.float32)
    t_sp = np.concatenate([inp["W_species"], z], 0).astype(np.float16)
    t_ab = np.concatenate([inp["W_ability"], z], 0).astype(np.float16)
    t_it = np.concatenate([inp["W_item"], z], 0).astype(np.float16)
    W_mv = inp["W_moveset"][:NUM_MOVES]
    W_pp = inp["W_moveset"][NUM_MOVES:]
    t_mv = np.concatenate(
        [np.concatenate([W_mv, W_pp], 1),
         np.zeros((1, 2 * D), np.float32)], 0).astype(np.float16)

    bias = (inp["b_species"] + inp["b_ability"] + inp["b_item"] + inp["b_moveset"]
            + inp["b_level"] + inp["b_hp"] + inp["b_volatiles"] + inp["b_feats"]
            + inp["b_bool"]).astype(np.float32)
    w_small = np.concatenate(
        [inp["W_level"], inp["W_hp"], inp["W_volatiles"], inp["W_feats"],
         inp["W_bool"], bias[None]], 0).astype(np.float16)
    assert w_small.shape == (K_DENSE, D)
    return t_sp, t_ab, t_it, t_mv, w_small


def _build_x_dense(e, mask):
    """[K_DENSE, n] fp16 dense-block encodings; masked columns zeroed."""
    n = e.shape[0]
    rows = [
        _bits_np(e[:, LEVEL], 7).T,                      # 7
        _bits_np(e[:, HP], 10).T,                        # 10
        _bits_np(e[:, VOL0:VOL_END], 4).reshape(n, 36).T,  # 36
        np.stack([e[:, LEVEL] / 100.0, e[:, HP] / 1023.0], 0).astype(np.float32),  # 2
        (e[:, BOOST0:BOOST0 + 7].T * 0.5).astype(np.float32),  # 7
    ]
    oh = np.zeros((125, n), np.float32)
    off = 0
    for col, k in [(GENDER, 4), (STATUS, 8), (CALLED_BACK, 2), (TRAPPED, 2),
                   (NEWLY_SW, 2), (TOXIC, 8), (SLEEP, 4), (FAINTED, 2), (ACTIVE, 2)]:
        oh[off + e[:, col], np.arange(n)] = 1.0
        off += k
    for i in range(7):
        oh[off + e[:, BOOST0 + i] + 6, np.arange(n)] = 1.0
        off += 13
    assert off == 125
    rows.append(oh)
    rows.append(np.ones((1, n), np.float32))
    x = np.concatenate(rows, 0)
    x[:, ~mask] = 0.0
    return x.astype(np.float16)


def _wrap_idx(idx):
    """[n] -> [128, n/16] wrapped layout: entry [p%16, s] = idx[s*16+p%16],
    replicated across the 8 16-partition stripes (one per Q7 core)."""
    w = np.ascontiguousarray(idx.astype(np.int16).reshape(-1, 16).T)
    return np.tile(w, (8, 1))


def _prep_core(e, mask):
    """Per-core derived arrays for one shard (e: [n, 39] int32)."""
    n = e.shape[0]
    zr_sp, zr_ab, zr_it, zr_mv = NUM_SPECIES, NUM_ABILITIES, NUM_ITEMS, NUM_MOVES
    sp = np.where(mask, e[:, SPECIES], zr_sp)
    ab = np.where(mask, e[:, ABILITY], zr_ab)
    it = np.where(mask, e[:, ITEM], zr_it)
    mv = np.where(mask[:, None], e[:, MOVEID0:MOVEPP0], zr_mv)  # [n,4]
    pp = np.where(mask[:, None], e[:, MOVEPP0:HAS_STATUS] / 1023.0, 0.0)  # [n,4]

    idx_sp = _wrap_idx(sp)
    idx_ab = _wrap_idx(ab)
    idx_it = _wrap_idx(it)
    # one tensor [16, 4*n/16]; move m occupies columns [m*n/16, (m+1)*n/16)
    idx_mv = np.concatenate([_wrap_idx(mv[:, m]) for m in range(4)], 1)
    # pp scalars: [128, 4, n/128], entry [p, m, G] = pp[G*128+p, m]
    pp_sc = np.ascontiguousarray(
        pp.reshape(n // 128, 128, 4).transpose(1, 2, 0)).astype(np.float32)
    x = _build_x_dense(e, mask)  # [188, E]
    return dict(idx_sp=idx_sp, idx_ab=idx_ab, idx_it=idx_it, idx_mv=idx_mv,
                pp_sc=pp_sc, x_hi=np.ascontiguousarray(x[:K_HI]),
                x_lo=np.ascontiguousarray(x[K_HI:]))


# ============================ bass program ============================

def _build_nc(n_ent):
    """SPMD program for one core processing n_ent entities."""
    n_chunks = n_ent // CHUNK
    nc = bacc.Bacc("TRN2", target_bir_lowering=False, debug=False)

    t_sp = nc.dram_tensor("t_sp", [NUM_SPECIES + 1, D], F16, kind="ExternalInput")
    t_ab = nc.dram_tensor("t_ab", [NUM_ABILITIES + 1, D], F16, kind="ExternalInput")
    t_it = nc.dram_tensor("t_it", [NUM_ITEMS + 1, D], F16, kind="ExternalInput")
    t_mv = nc.dram_tensor("t_mv", [NUM_MOVES + 1, 2 * D], F16, kind="ExternalInput")
    w_hi_d = nc.dram_tensor("w_hi", [K_HI, D], F16, kind="ExternalInput")
    w_lo_d = nc.dram_tensor("w_lo", [K_LO, D], F16, kind="ExternalInput")
    x_hi_d = nc.dram_tensor("x_hi", [K_HI, n_ent], F16, kind="ExternalInput")
    x_lo_d = nc.dram_tensor("x_lo", [K_LO, n_ent], F16, kind="ExternalInput")
    idx_sp_d = nc.dram_tensor("idx_sp", [128, n_ent // 16], I16, kind="ExternalInput")
    idx_ab_d = nc.dram_tensor("idx_ab", [128, n_ent // 16], I16, kind="ExternalInput")
    idx_it_d = nc.dram_tensor("idx_it", [128, n_ent // 16], I16, kind="ExternalInput")
    idx_mv_d = nc.dram_tensor("idx_mv", [128, 4 * n_ent // 16], I16, kind="ExternalInput")
    pp_sc_d = nc.dram_tensor("pp_sc", [128, 4, n_ent // 128], F32, kind="ExternalInput")
    out_d = nc.dram_tensor("out", [n_ent, D], F32, kind="ExternalOutput")

    # dram view [chunk, p, g, d] matching gather partition layout
    out_r = out_d.ap().rearrange("(c g p) d -> c p g d", p=128, g=GROUPS)

    scol = n_ent // 16  # wrapped-idx columns per unit

    with tile.TileContext(nc) as tc:
        with (
            tc.tile_pool(name="consts", bufs=1) as pc,
            tc.tile_pool(name="gather", bufs=1) as pg,
            tc.tile_pool(name="accp", bufs=2) as pa,
            tc.tile_pool(name="outp", bufs=2) as po,
            tc.tile_pool(name="psum", bufs=8, space="PSUM") as pp_pool,
        ):
            # ---- constants into SBUF ----
            w_hi = pc.tile([K_HI, D], F16, tag="w_hi")
            w_lo = pc.tile([K_LO, D], F16, tag="w_lo")
            x_hi = pc.tile([K_HI, n_ent], F16, tag="x_hi")
            x_lo = pc.tile([K_LO, n_ent], F16, tag="x_lo")
            idx_sp = pc.tile([128, scol], I16, tag="idx_sp")
            idx_ab = pc.tile([128, scol], I16, tag="idx_ab")
            idx_it = pc.tile([128, scol], I16, tag="idx_it")
            idx_mv = pc.tile([128, 4 * scol], I16, tag="idx_mv")
            pp_sc = pc.tile([128, 4, n_ent // 128], F32, tag="pp_sc")
            ident = pc.tile([128, 128], F16, tag="ident")

            nc.sync.dma_start(w_hi[:], w_hi_d.ap())
            nc.sync.dma_start(w_lo[:], w_lo_d.ap())
            nc.sync.dma_start(x_hi[:], x_hi_d.ap())
            nc.sync.dma_start(x_lo[:], x_lo_d.ap())
            nc.sync.dma_start(idx_sp[:], idx_sp_d.ap())
            nc.sync.dma_start(idx_ab[:], idx_ab_d.ap())
            nc.sync.dma_start(idx_it[:], idx_it_d.ap())
            nc.sync.dma_start(idx_mv[:], idx_mv_d.ap())
            nc.sync.dma_start(pp_sc[:], pp_sc_d.ap())

            nc.gpsimd.memset(ident[:], 0.0)
            nc.gpsimd.affine_select(
                out=ident[:], in_=ident[:],
                compare_op=mybir.AluOpType.not_equal,
                fill=1.0, base=0, pattern=[[-1, 128]], channel_multiplier=1,
            )

            ccol = CHUNK // 16  # wrapped-idx columns per chunk

            for c in range(n_chunks):
                isl = slice(c * ccol, (c + 1) * ccol)

                def gath(dst, table_ap, idx_ap, estep):
                    nc.gpsimd.dma_gather(
                        dst[:], table_ap, idx_ap, CHUNK, CHUNK, D,
                        elem_step=estep)

                g_sp = pg.tile([128, GROUPS, D], F16, tag="g_sp")
                gath(g_sp, t_sp.ap(), idx_sp[:, isl], D)
                g_ab = pg.tile([128, GROUPS, D], F16, tag="g_ab")
                gath(g_ab, t_ab.ap(), idx_ab[:, isl], D)
                g_it = pg.tile([128, GROUPS, D], F16, tag="g_it")
                gath(g_it, t_it.ap(), idx_it[:, isl], D)
                g_mv = []
                g_pp = []
                for m in range(4):
                    misl = slice(m * scol + c * ccol, m * scol + (c + 1) * ccol)
                    gq = pg.tile([128, GROUPS, 2 * D], F16, tag=f"g_q{m}")
                    nc.gpsimd.dma_gather(
                        gq[:], t_mv.ap(), idx_mv[:, misl], CHUNK, CHUNK,
                        2 * D, elem_step=2 * D)
                    g_mv.append(gq[:, :, 0:D])
                    g_pp.append(gq[:, :, D:2 * D])

                # ---- DVE accumulator: species + ability + item + pp terms ----
                acc = pa.tile([128, GROUPS, D], F16, tag="acc")
                nc.vector.tensor_tensor(acc[:], g_sp[:], g_ab[:], mybir.AluOpType.add)
                nc.vector.tensor_tensor(acc[:], acc[:], g_it[:], mybir.AluOpType.add)
                for m in range(4):
                    for g in range(GROUPS):
                        G = c * GROUPS + g
                        nc.vector.scalar_tensor_tensor(
                            acc[:, g, :], g_pp[m][:, g, :],
                            pp_sc[:, m, G:G + 1], acc[:, g, :],
                            mybir.AluOpType.mult, mybir.AluOpType.add)

                # ---- PE: dense block + move-halves + acc into PSUM ----
                out_sb = po.tile([128, GROUPS, D], F16, tag="out_sb")
                for g in range(GROUPS):
                    ecol = c * CHUNK + g * 128
                    ps = pp_pool.tile([128, D], F32, tag="ps")
                    nc.tensor.matmul(ps[:], x_hi[:, ecol:ecol + 128], w_hi[:],
                                     start=True, stop=False)
                    nc.tensor.matmul(ps[:], x_lo[:, ecol:ecol + 128], w_lo[:],
                                     start=False, stop=False)
                    for m in range(4):
                        nc.tensor.matmul(ps[:], ident[:], g_mv[m][:, g, :],
                                         start=False, stop=False)
                    nc.tensor.matmul(ps[:], ident[:], acc[:, g, :],
                                     start=False, stop=True)
                    nc.scalar.activation(out_sb[:, g, :], ps[:],
                                         mybir.ActivationFunctionType.Copy)

                nc.gpsimd.dma_start(out_r[c], out_sb[:])

    nc.compile()
    return nc


# ============================ entry point ============================

def _get_nc(n_ent):
    if n_ent not in _NC_CACHE:
        _NC_CACHE[n_ent] = _build_nc(n_ent)
    return _NC_CACHE[n_ent]


def kernel(**inputs):
    entity = np.asarray(inputs["entity"])
    mask_full = ~((entity[:, SPECIES] == 0) | (entity[:, SPECIES] == 1))

    t_sp, t_ab, t_it, t_mv, w_small = _build_tables(
        {k: np.asarray(v) for k, v in inputs.items()})
    shared = dict(t_sp=t_sp, t_ab=t_ab, t_it=t_it, t_mv=t_mv,
                  w_hi=np.ascontiguousarray(w_small[:K_HI]),
                  w_lo=np.ascontiguousarray(w_small[K_HI:]))

    in_maps = []
    for c in range(N_CORES):
        sl = slice(c * E, (c + 1) * E)
        per = _prep_core(entity[sl], mask_full[sl])
        in_maps.append({**shared, **per})

    nc = _get_nc(E)
    res = run_bass_kernel_spmd(nc, in_maps, list(range(N_CORES)))
    global LAST_RESULTS
    LAST_RESULTS = res
    emb = np.concatenate([r["out"] for r in res.results], 0)
    return emb.astype(np.float32), mask_full
